# revision 5
# baseline (speedup 1.0000x reference)
"""ColBERT MaxSim kernel for Trainium2 (Bass/Tile), data-parallel over batch.

Problem shapes (hardcoded):
  Q_emb [128, 32, 768] f32, D_emb [128, 1024, 768] f32,
  doc_input_ids [128, 1024] i32, doc_attn_mask [128, 1024] i32,
  W [128, 768] f32  ->  out [128] f32

Math (reference):
  Q = l2norm(Q_emb @ W.T);  D = l2norm((D_emb @ W.T) * (ids != 0))
  score[b,q,t] = -(|Qn|^2 + |Dn|^2 - 2 Qn.Dn), NEG where attn==0
  out[b] = sum_q max_t score

Reformulation used on device (per batch):
  out[b] = sum_q [ max_t ( a[t] * (Qn[q] . Dp[t]) + c[t] ) ] - sum_q |Qn[q]|^2
  where Dp = unnormalized projection, dinv[t] = 1/max(sqrt(|Dp[t]|^2), eps),
        a[t] = 2*keep*punct*dinv[t],
        c[t] = keep ? (punct ? -|Dp[t]|^2*dinv^2 : 0) : NEG

Sharding: batch dim 128 -> 8 cores x 16 batches. Host pre-transposes
D_emb/Q_emb into [H-on-partition] layouts so all device DMAs are contiguous.
"""

import os
import sys
from contextlib import ExitStack

import numpy as np

sys.path.insert(0, "/opt/trn_rl_repo")

import concourse.bass as bass
from concourse import bacc
import concourse.tile as tile
from concourse import mybir
from concourse.bass_utils import run_bass_kernel_spmd

B, QL, DL, H, DIM = 128, 32, 1024, 768, 128
NC, BPC = 8, B // 8          # cores, batches per core
HC = H // 128                # 6 h-chunks
TT = DL // 128               # 8 token chunks per batch
NG = BPC // 4                # groups of 4 batches
EPS = 1e-12
NEG = -100000.0

F32 = mybir.dt.float32
F32R = mybir.dt.float32r

_CACHE = {}


def _build_kernel(proj_dtype=F32R, trace_label=""):
    key = (proj_dtype, trace_label)
    if key in _CACHE:
        return _CACHE[key]

    nc = bacc.Bacc("TRN2", target_bir_lowering=False)

    dt_d = nc.dram_tensor("dt_in", [BPC, 128, HC * DL], proj_dtype, kind="ExternalInput")
    qt_d = nc.dram_tensor("qt_in", [NG, 128, HC * 128], F32, kind="ExternalInput")
    wtr_d = nc.dram_tensor("wtr_in", [128, H], proj_dtype, kind="ExternalInput")
    wtf_d = nc.dram_tensor("wtf_in", [128, H], F32, kind="ExternalInput")
    vk2_d = nc.dram_tensor("vk2_in", [128, BPC * TT], F32, kind="ExternalInput")
    cb_d = nc.dram_tensor("cb_in", [128, BPC * TT], F32, kind="ExternalInput")
    cst_d = nc.dram_tensor("cst_in", [128, 133], F32, kind="ExternalInput")
    out_d = nc.dram_tensor("out", [1, BPC], F32, kind="ExternalOutput")

    with tile.TileContext(nc) as tc, ExitStack() as ctx:
        const = ctx.enter_context(tc.tile_pool(name="const", bufs=1))
        qpool = ctx.enter_context(tc.tile_pool(name="qpool", bufs=2))
        dpool = ctx.enter_context(tc.tile_pool(name="dpool", bufs=3))
        spool = ctx.enter_context(tc.tile_pool(name="spool", bufs=2))
        hpool = ctx.enter_context(tc.tile_pool(name="hpool", bufs=2))
        smalls = ctx.enter_context(tc.tile_pool(name="smalls", bufs=4))
        ps_p = ctx.enter_context(tc.tile_pool(name="ps_p", bufs=2, space="PSUM"))
        ps_d = ctx.enter_context(tc.tile_pool(name="ps_d", bufs=2, space="PSUM"))
        ps_s = ctx.enter_context(tc.tile_pool(name="ps_s", bufs=2, space="PSUM"))
        ps_q = ctx.enter_context(tc.tile_pool(name="ps_q", bufs=2, space="PSUM"))

        # ---- constants ----
        wtr = const.tile([128, H], proj_dtype)
        nc.sync.dma_start(wtr, wtr_d[:, :])
        wtf = const.tile([128, H], F32)
        nc.sync.dma_start(wtf, wtf_d[:, :])
        vk2 = const.tile([128, BPC * TT], F32)
        nc.sync.dma_start(vk2, vk2_d[:, :])
        cb = const.tile([128, BPC * TT], F32)
        nc.sync.dma_start(cb, cb_d[:, :])
        cst = const.tile([128, 133], F32)
        nc.sync.dma_start(cst, cst_d[:, :])
        bo = cst[:, 0:4]        # block-ones [128, 4]
        ones = cst[:, 4:5]      # ones [128, 1]
        ident = cst[:, 5:133]   # identity [128, 128]
        qt_all = const.tile([128, NG * HC * 128], F32)
        for g in range(NG):
            nc.sync.dma_start(qt_all[:, g * H: (g + 1) * H], qt_d[g])

        qnt_all = const.tile([128, NG * 128], F32)   # [d, g*128 + bi*32 + q]
        qsq_all = const.tile([128, NG], F32)
        out_sb = const.tile([1, BPC], F32)

        # ---- Q stage: 4 batches per group ----
        for g in range(NG):
            qp_ps = ps_q.tile([128, 128], F32, tag="psq")
            for c in range(HC):
                nc.tensor.matmul(
                    qp_ps,
                    lhsT=qt_all[:, g * H + c * 128: g * H + (c + 1) * 128],
                    rhs=wtf[:, c * 128:(c + 1) * 128],
                    start=(c == 0),
                    stop=(c == HC - 1),
                )
            qp_sb = qpool.tile([128, 128], F32, tag="qp")
            nc.vector.tensor_copy(qp_sb, qp_ps)
            sq_dump = qpool.tile([128, 128], F32, tag="sqd")
            qs_raw = smalls.tile([128, 1], F32, tag="qs")
            nc.scalar.activation(
                sq_dump, qp_ps, mybir.ActivationFunctionType.Square,
                accum_out=qs_raw,
            )
            t1 = smalls.tile([128, 1], F32, tag="t1")
            nc.scalar.sqrt(t1, qs_raw)
            t2 = smalls.tile([128, 1], F32, tag="t2")
            nc.vector.tensor_scalar_max(t2, t1, EPS)
            qrinv = smalls.tile([128, 1], F32, tag="qrinv")
            nc.vector.reciprocal(qrinv, t2)
            qn = qpool.tile([128, 128], F32, tag="qn")
            nc.vector.tensor_scalar_mul(qn, qp_sb, qrinv)
            u = smalls.tile([128, 1], F32, tag="uq")
            nc.vector.tensor_scalar_mul(u, qs_raw, qrinv)
            nc.vector.tensor_scalar_mul(qsq_all[:, g:g + 1], u, qrinv)
            qnt_ps = ps_q.tile([128, 128], F32, tag="psq")
            nc.tensor.transpose(qnt_ps, qn, ident)
            nc.vector.tensor_copy(qnt_all[:, g * 128:(g + 1) * 128], qnt_ps)

        # ---- D loop ----
        m4 = None
        for b in range(BPC):
            g, bi = b // 4, b % 4
            dt = dpool.tile([128, HC * DL], proj_dtype, tag="dt")
            nc.sync.dma_start(dt, dt_d[b])

            dpt = spool.tile([128, DL], F32, tag="dpt")
            sqt = spool.tile([128, DL], F32, tag="sqt")
            for th in range(2):
                pp = ps_p.tile([128, 512], F32, tag="pp")
                for c in range(HC):
                    nc.tensor.matmul(
                        pp,
                        lhsT=wtr[:, c * 128:(c + 1) * 128],
                        rhs=dt[:, c * DL + th * 512: c * DL + th * 512 + 512],
                        start=(c == 0),
                        stop=(c == HC - 1),
                    )
                nc.vector.tensor_copy(dpt[:, th * 512:(th + 1) * 512], pp)
                nc.scalar.square(sqt[:, th * 512:(th + 1) * 512], pp)

            dsq_ps = ps_s.tile([128, TT], F32, tag="dsq")
            for tt in range(TT):
                nc.tensor.matmul(
                    dsq_ps[:, tt:tt + 1],
                    lhsT=sqt[:, tt * 128:(tt + 1) * 128],
                    rhs=ones,
                    start=True, stop=True,
                )
            dsq = smalls.tile([128, TT], F32, tag="dsqs")
            nc.vector.tensor_copy(dsq, dsq_ps)
            s1 = smalls.tile([128, TT], F32, tag="s1")
            nc.scalar.sqrt(s1, dsq)
            s2 = smalls.tile([128, TT], F32, tag="s2")
            nc.vector.tensor_scalar_max(s2, s1, EPS)
            dinv = smalls.tile([128, TT], F32, tag="dinv")
            nc.vector.reciprocal(dinv, s2)
            a_t = smalls.tile([128, TT], F32, tag="a_t")
            nc.vector.tensor_tensor(
                a_t, vk2[:, b * TT:(b + 1) * TT], dinv, mybir.AluOpType.mult)
            u1 = smalls.tile([128, TT], F32, tag="u1")
            nc.vector.tensor_tensor(u1, dsq, dinv, mybir.AluOpType.mult)
            u2 = smalls.tile([128, TT], F32, tag="u2")
            nc.vector.tensor_tensor(u2, u1, dinv, mybir.AluOpType.mult)
            v = smalls.tile([128, TT], F32, tag="v")
            nc.vector.tensor_tensor(
                v, u2, vk2[:, b * TT:(b + 1) * TT], mybir.AluOpType.mult)
            c_t = smalls.tile([128, TT], F32, tag="c_t")
            nc.vector.scalar_tensor_tensor(
                c_t, v, -0.5, cb[:, b * TT:(b + 1) * TT],
                mybir.AluOpType.mult, mybir.AluOpType.add)

            qd_ps = ps_d.tile([128, TT * QL], F32, tag="qd")
            for tt in range(TT):
                nc.tensor.matmul(
                    qd_ps[:, tt * QL:(tt + 1) * QL],
                    lhsT=dpt[:, tt * 128:(tt + 1) * 128],
                    rhs=qnt_all[:, b * QL:(b + 1) * QL],
                    start=True, stop=True,
                )
            ht = hpool.tile([128, TT * QL], F32, tag="ht")
            for tt in range(TT):
                nc.vector.tensor_scalar(
                    ht[:, tt * QL:(tt + 1) * QL],
                    qd_ps[:, tt * QL:(tt + 1) * QL],
                    a_t[:, tt:tt + 1],
                    c_t[:, tt:tt + 1],
                    mybir.AluOpType.mult,
                    mybir.AluOpType.add,
                )
            if bi == 0:
                m4 = qpool.tile([128, 4 * QL], F32, tag="m4")
            m1 = hpool.tile([128, 4 * QL], F32, tag="m1")
            nc.vector.tensor_tensor(
                m1, ht[:, 0:128], ht[:, 128:256], mybir.AluOpType.max)
            m2 = hpool.tile([128, 2 * QL], F32, tag="m2")
            nc.vector.tensor_tensor(
                m2, m1[:, 0:64], m1[:, 64:128], mybir.AluOpType.max)
            nc.vector.tensor_tensor(
                m4[:, bi * QL:(bi + 1) * QL], m2[:, 0:QL], m2[:, QL:2 * QL],
                mybir.AluOpType.max)

            if bi == 3:
                mt_ps = ps_q.tile([128, 128], F32, tag="psq")
                nc.tensor.transpose(mt_ps, m4, ident)
                mq = smalls.tile([128, 1], F32, tag="mq")
                nc.vector.reduce_max(mq, mt_ps, axis=mybir.AxisListType.X,
                                     op=mybir.AluOpType.max)
                r = smalls.tile([128, 1], F32, tag="r")
                nc.vector.tensor_tensor(
                    r, mq, qsq_all[:, g:g + 1], mybir.AluOpType.subtract)
                s_ps = ps_s.tile([1, 4], F32, tag="dsq")
                nc.tensor.matmul(s_ps, lhsT=r, rhs=bo, start=True, stop=True)
                nc.vector.tensor_copy(out_sb[:, g * 4:(g + 1) * 4], s_ps)

        nc.sync.dma_start(out_d[:, :], out_sb)

    nc.finalize()
    _CACHE[key] = nc
    return nc


def _prep_host(inputs):
    """Full-input -> per-core in_maps."""
    Q_emb = np.ascontiguousarray(inputs["Q_emb"], dtype=np.float32)
    D_emb = np.ascontiguousarray(inputs["D_emb"], dtype=np.float32)
    ids = np.asarray(inputs["doc_input_ids"])
    attn = np.asarray(inputs["doc_attn_mask"])
    W = np.ascontiguousarray(inputs["W"], dtype=np.float32)

    keep = (attn != 0)
    punct = (ids != 0)
    vk2 = (2.0 * keep * punct).astype(np.float32)                 # [B, DL]
    cbase = np.where(keep, 0.0, NEG).astype(np.float32)           # [B, DL]

    wt = np.ascontiguousarray(
        W.reshape(DIM, HC, 128).transpose(2, 1, 0).reshape(128, H))

    cst = np.zeros((128, 133), np.float32)
    for j in range(128):
        cst[j, j // 32] = 1.0
    cst[:, 4] = 1.0
    cst[:, 5:133] = np.eye(128, dtype=np.float32)

    in_maps = []
    for ci in range(NC):
        bs = slice(ci * BPC, (ci + 1) * BPC)
        DT = np.ascontiguousarray(
            D_emb[bs].reshape(BPC, DL, HC, 128).transpose(0, 3, 2, 1)
        ).reshape(BPC, 128, HC * DL)
        QT = np.ascontiguousarray(
            Q_emb[bs].reshape(NG, 4, QL, HC, 128).transpose(0, 4, 3, 1, 2)
        ).reshape(NG, 128, HC * 128)
        VK2 = np.ascontiguousarray(
            vk2[bs].reshape(BPC, TT, 128).transpose(2, 0, 1)
        ).reshape(128, BPC * TT)
        CB = np.ascontiguousarray(
            cbase[bs].reshape(BPC, TT, 128).transpose(2, 0, 1)
        ).reshape(128, BPC * TT)
        in_maps.append({
            "dt_in": DT,
            "qt_in": QT,
            "wtr_in": wt,
            "wtf_in": wt,
            "vk2_in": VK2,
            "cb_in": CB,
            "cst_in": cst,
        })
    return in_maps


def kernel(Q_emb, D_emb, doc_input_ids, doc_attn_mask, W, _trace=False,
           _proj_dtype=None, _tmpdir=None):
    proj_dtype = _proj_dtype if _proj_dtype is not None else F32R
    nc = _build_kernel(proj_dtype=proj_dtype)
    in_maps = _prep_host({
        "Q_emb": Q_emb, "D_emb": D_emb, "doc_input_ids": doc_input_ids,
        "doc_attn_mask": doc_attn_mask, "W": W,
    })
    res = run_bass_kernel_spmd(
        nc, in_maps, core_ids=list(range(NC)),
        trace=_trace, tmpdir=_tmpdir,
        **({"trace_cores": list(range(NC)), "stitch_traces": False}
           if _trace else {}),
    )
    out = np.concatenate([r["out"].reshape(BPC) for r in res.results])
    if _trace:
        kernel._last_results = res
    return out.astype(np.float32)
